# revision 2
# baseline (speedup 1.0000x reference)
"""Mixed-precision binary conv v3: image-granular DMAs + fp8-DR tap pairs.

y = conv2d(x, sign(w)): x (32,128,56,56) f32, w (256,128,3,3) -> y (32,256,56,56).
8 cores x 4 images.

Per (row-tile, co-half): 2 fp8e4 DoubleRow MMs (taps (0,0),(0,1) and
(1,0),(1,1), one MM per pair) + 5 bf16 MMs (remaining taps). HW-measured
per-MM cost at N=448 is ~134 ns for both kinds -> MM floor ~52 us/core.

All DMAs are whole-image contiguous transfers (per-tile DMAs made v2
DMA-bound): padded bf16 image in, 2-plane shifted fp8 copy in, staged
[128, 3136] f32 out per (image, co-half). ~17 DMAs per pass, 20.1 MB.

Mixed-tap quantization rel err (vs f32 reference): 1.78e-2.
"""

import numpy as np
import ml_dtypes

import concourse.bass as bass
import concourse.bacc as bacc
import concourse.mybir as mybir
import concourse.tile as tile
from concourse.bass_utils import run_bass_kernel_spmd

N_CORES = 8
B, CI, H, W = 32, 128, 56, 56
CO = 256
BPC = B // N_CORES
HP, WP = H + 2, W + 2      # 58 padded rows / cols
W8 = 64                    # fp8 plane row pitch (aligned)
ROWS_PER_TILE = 8
N_ROW_TILES = H // ROWS_PER_TILE   # 7
NFREE = ROWS_PER_TILE * W          # 448
HW_FLAT = H * W                    # 3136

F32 = mybir.dt.float32
BF16 = mybir.dt.bfloat16
FP8 = mybir.dt.float8e4

NG = 2                               # DR pair groups: kh=0 and kh=1 (kw 0,1)
BF_TAPS = [(0, 2), (1, 2), (2, 0), (2, 1), (2, 2)]

DEFAULT_CFG = dict(psum_bufs=6, out_bufs=6, xb_bufs=2, x8_bufs=2, y_bufs=4)


def _emit_body(nc, pools, x8_d, xb_d, y_d, w_dr_sb, w_bf_sb, cfg):
    xb_pool, x8_pool, out_pool, psum_pool, y_pool = pools
    for n in range(BPC):
        xb = xb_pool.tile([CI, HP, WP], BF16, name="xb")
        x8 = x8_pool.tile([CI, NG, HP, W8], FP8, name="x8")
        nc.sync.dma_start(xb[:], xb_d[n])
        nc.sync.dma_start(x8[:], x8_d[n])
        for co_half in range(CO // 128):
            cs = slice(co_half * 128, (co_half + 1) * 128)
            ystage = y_pool.tile([128, HW_FLAT], F32, name="ystage")
            for t in range(N_ROW_TILES):
                r0 = t * ROWS_PER_TILE
                ps = psum_pool.tile([128, NFREE], F32, name="ps")
                for g in range(NG):
                    rhs = x8[:, :, r0 + g:r0 + g + ROWS_PER_TILE, 0:W]
                    nc.tensor.matmul(
                        ps[:], w_dr_sb[:, g, :, cs], rhs,
                        start=(g == 0), stop=False,
                        perf_mode=mybir.MatmulPerfMode.DoubleRow,
                    )
                for j, (kh, kw) in enumerate(BF_TAPS):
                    rhs = xb[:, r0 + kh:r0 + kh + ROWS_PER_TILE, kw:kw + W]
                    nc.tensor.matmul(
                        ps[:], w_bf_sb[:, j, cs], rhs,
                        start=False, stop=(j == len(BF_TAPS) - 1),
                    )
                nc.vector.tensor_copy(
                    ystage[:, t * NFREE:(t + 1) * NFREE], ps[:])
            nc.sync.dma_start(
                y_d[n, cs].rearrange("c h w -> c (h w)"), ystage[:])


def build_program(static_reps: int = 1, **overrides) -> bass.Bass:
    cfg = dict(DEFAULT_CFG, **overrides)
    nc = bacc.Bacc(name="binconv_v3")
    xb_d = nc.dram_tensor("xb", (BPC, CI, HP, WP), BF16, kind="ExternalInput")
    x8_d = nc.dram_tensor("x8", (BPC, CI, NG, HP, W8), FP8, kind="ExternalInput")
    w_dr_d = nc.dram_tensor("w_dr", (NG, CI, 2, CO), FP8, kind="ExternalInput")
    w_bf_d = nc.dram_tensor("w_bf", (len(BF_TAPS), CI, CO), BF16,
                            kind="ExternalInput")
    y_d = nc.dram_tensor("y", (BPC, CO, H, W), F32, kind="ExternalOutput")

    with tile.TileContext(nc) as tc:
        with (
            tc.tile_pool(name="wpool", bufs=1) as wpool,
            tc.tile_pool(name="xbp", bufs=cfg["xb_bufs"]) as xb_pool,
            tc.tile_pool(name="x8p", bufs=cfg["x8_bufs"]) as x8_pool,
            tc.tile_pool(name="outb", bufs=cfg["out_bufs"]) as out_pool,
            tc.tile_pool(name="yst", bufs=cfg["y_bufs"]) as y_pool,
            tc.tile_pool(name="psum", bufs=cfg["psum_bufs"], space="PSUM") as psum_pool,
        ):
            w_dr_sb = wpool.tile([CI, NG, 2, CO], FP8)
            w_bf_sb = wpool.tile([CI, len(BF_TAPS), CO], BF16)
            nc.sync.dma_start(w_dr_sb[:], w_dr_d[:].rearrange("g p two c -> p g two c"))
            nc.sync.dma_start(w_bf_sb[:], w_bf_d[:].rearrange("k p c -> p k c"))
            pools = (xb_pool, x8_pool, out_pool, psum_pool, y_pool)
            for _ in range(static_reps):
                _emit_body(nc, pools, x8_d, xb_d, y_d, w_dr_sb, w_bf_sb, cfg)

    nc.finalize()
    return nc


def prep_weights(weights: np.ndarray):
    bw = np.sign(np.asarray(weights, dtype=np.float32))  # (co, ci, kh, kw)
    w_dr = np.empty((NG, CI, 2, CO), dtype=ml_dtypes.float8_e4m3)
    for g in range(NG):
        for j in range(2):
            w_dr[g, :, j, :] = bw[:, :, g, j].T.astype(ml_dtypes.float8_e4m3)
    w_bf = np.empty((len(BF_TAPS), CI, CO), dtype=ml_dtypes.bfloat16)
    for j, (kh, kw) in enumerate(BF_TAPS):
        w_bf[j] = bw[:, :, kh, kw].T.astype(ml_dtypes.bfloat16)
    return w_dr, w_bf


def prep_x(x: np.ndarray):
    x = np.asarray(x, dtype=np.float32)
    xpad = np.zeros((B, CI, HP, WP), dtype=np.float32)
    xpad[:, :, 1:H + 1, 1:W + 1] = x
    xb = xpad.astype(ml_dtypes.bfloat16)
    # fp8 shifted planes: plane j at (row, c) = xpad[row, c+j]; cols >= WP-j zero
    x8 = np.zeros((B, CI, NG, HP, W8), dtype=ml_dtypes.float8_e4m3)
    xpad8 = xpad.astype(ml_dtypes.float8_e4m3)
    for j in range(2):
        x8[:, :, j, :, 0:WP - j] = xpad8[:, :, :, j:WP]
    return xb, x8


def make_in_maps(x: np.ndarray, weights: np.ndarray) -> list[dict]:
    xb, x8 = prep_x(x)
    w_dr, w_bf = prep_weights(weights)
    return [
        {"xb": xb[i * BPC:(i + 1) * BPC], "x8": x8[i * BPC:(i + 1) * BPC],
         "w_dr": w_dr, "w_bf": w_bf}
        for i in range(N_CORES)
    ]


def _run_once(x, weights) -> np.ndarray:
    nc = build_program()
    in_maps = make_in_maps(x, weights)
    res = run_bass_kernel_spmd(nc, in_maps, core_ids=list(range(N_CORES)))
    return np.concatenate([r["y"] for r in res.results], axis=0)


_SUBPROC_SRC = """
import sys, numpy as np
sys.path.insert(0, sys.argv[1])
import kernel as K
x = np.load(sys.argv[2]); w = np.load(sys.argv[3])
np.save(sys.argv[4], K._run_once(x, w))
"""


def kernel(x, weights) -> np.ndarray:
    x = np.ascontiguousarray(np.asarray(x, dtype=np.float32))
    weights = np.ascontiguousarray(np.asarray(weights, dtype=np.float32))
    try:
        return _run_once(x, weights)
    except Exception as first_exc:
        # Transient device wedges (NRT_EXEC_UNIT_UNRECOVERABLE, mesh desync)
        # poison the in-process PJRT client; only a fresh process recovers.
        import os
        import subprocess
        import sys
        import tempfile
        import time

        last_exc = first_exc
        moddir = os.path.dirname(os.path.abspath(__file__))
        for attempt in range(2):
            time.sleep(10 * (attempt + 1))
            try:
                with tempfile.TemporaryDirectory() as td:
                    xp, wp, yp = (os.path.join(td, f) for f in
                                  ("x.npy", "w.npy", "y.npy"))
                    np.save(xp, x)
                    np.save(wp, weights)
                    subprocess.run(
                        [sys.executable, "-c", _SUBPROC_SRC, moddir, xp, wp, yp],
                        check=True, timeout=900,
                    )
                    return np.load(yp)
            except Exception as e:
                last_exc = e
        raise last_exc


# revision 5
# speedup vs baseline: 1.1765x; 1.1765x over previous
"""Mixed-precision binary conv v3: image-granular DMAs + fp8-DR tap pairs.

y = conv2d(x, sign(w)): x (32,128,56,56) f32, w (256,128,3,3) -> y (32,256,56,56).
8 cores x 4 images.

Per (row-tile, co-half): 2 fp8e4 DoubleRow MMs (taps (0,0),(0,1) and
(1,0),(1,1), one MM per pair) + 5 bf16 MMs (remaining taps). HW-measured
per-MM cost at N=448 is ~134 ns for both kinds -> MM floor ~52 us/core.

All DMAs are whole-image contiguous transfers (per-tile DMAs made v2
DMA-bound): padded bf16 image in, 2-plane shifted fp8 copy in, staged
[128, 3136] f32 out per (image, co-half). ~17 DMAs per pass, 20.1 MB.

Mixed-tap quantization rel err (vs f32 reference): 1.78e-2.
"""

import numpy as np
import ml_dtypes

import concourse.bass as bass
import concourse.bacc as bacc
import concourse.mybir as mybir
import concourse.tile as tile
from concourse.bass_utils import run_bass_kernel_spmd

N_CORES = 8
B, CI, H, W = 32, 128, 56, 56
CO = 256
BPC = B // N_CORES
HP, WP = H + 2, W + 2      # 58 padded rows / cols
W8 = 64                    # fp8 plane row pitch (aligned)
ROWS_PER_TILE = 8
N_ROW_TILES = H // ROWS_PER_TILE   # 7
NFREE = ROWS_PER_TILE * W          # 448
HW_FLAT = H * W                    # 3136

F32 = mybir.dt.float32
BF16 = mybir.dt.bfloat16
FP8 = mybir.dt.float8e4

NG = 2                               # DR pair groups: kh=0 and kh=1 (kw 0,1)
BF_TAPS = [(0, 2), (1, 2), (2, 0), (2, 1), (2, 2)]

DEFAULT_CFG = dict(psum_bufs=7, out_bufs=6, xb_bufs=3, x8_bufs=3, y_bufs=6,
                   y_ring="sync", copy_engine="vector")


def _emit_body(nc, pools, x8_d, xb_d, y_d, w_dr_sb, w_bf_sb, cfg):
    xb_pool, x8_pool, out_pool, psum_pool, y_pool = pools
    y_eng = nc.scalar if cfg["y_ring"] == "scalar" else nc.sync
    for n in range(BPC):
        xb = xb_pool.tile([CI, HP, WP], BF16, name="xb")
        x8 = x8_pool.tile([CI, NG, HP, W8], FP8, name="x8")
        nc.sync.dma_start(xb[:], xb_d[n])
        nc.sync.dma_start(x8[:], x8_d[n])
        for co_half in range(CO // 128):
            cs = slice(co_half * 128, (co_half + 1) * 128)
            ystage = y_pool.tile([128, HW_FLAT], F32, name="ystage")
            for t in range(N_ROW_TILES):
                r0 = t * ROWS_PER_TILE
                ps = psum_pool.tile([128, NFREE], F32, name="ps")
                for g in range(NG):
                    rhs = x8[:, :, r0 + g:r0 + g + ROWS_PER_TILE, 0:W]
                    nc.tensor.matmul(
                        ps[:], w_dr_sb[:, g, :, cs], rhs,
                        start=(g == 0), stop=False,
                        perf_mode=mybir.MatmulPerfMode.DoubleRow,
                    )
                for j, (kh, kw) in enumerate(BF_TAPS):
                    rhs = xb[:, r0 + kh:r0 + kh + ROWS_PER_TILE, kw:kw + W]
                    nc.tensor.matmul(
                        ps[:], w_bf_sb[:, j, cs], rhs,
                        start=False, stop=(j == len(BF_TAPS) - 1),
                    )
                ce = cfg["copy_engine"]
                if ce == "alternate":
                    ce = "vector" if t % 2 == 0 else "scalar"
                if ce == "scalar":
                    nc.scalar.copy(ystage[:, t * NFREE:(t + 1) * NFREE], ps[:])
                else:
                    nc.vector.tensor_copy(
                        ystage[:, t * NFREE:(t + 1) * NFREE], ps[:])
            y_eng.dma_start(
                y_d[n, cs].rearrange("c h w -> c (h w)"), ystage[:])


def build_program(static_reps: int = 1, **overrides) -> bass.Bass:
    cfg = dict(DEFAULT_CFG, **overrides)
    nc = bacc.Bacc(name="binconv_v3")
    xb_d = nc.dram_tensor("xb", (BPC, CI, HP, WP), BF16, kind="ExternalInput")
    x8_d = nc.dram_tensor("x8", (BPC, CI, NG, HP, W8), FP8, kind="ExternalInput")
    w_dr_d = nc.dram_tensor("w_dr", (NG, CI, 2, CO), FP8, kind="ExternalInput")
    w_bf_d = nc.dram_tensor("w_bf", (len(BF_TAPS), CI, CO), BF16,
                            kind="ExternalInput")
    y_d = nc.dram_tensor("y", (BPC, CO, H, W), F32, kind="ExternalOutput")

    with tile.TileContext(nc) as tc:
        with (
            tc.tile_pool(name="wpool", bufs=1) as wpool,
            tc.tile_pool(name="xbp", bufs=cfg["xb_bufs"]) as xb_pool,
            tc.tile_pool(name="x8p", bufs=cfg["x8_bufs"]) as x8_pool,
            tc.tile_pool(name="outb", bufs=cfg["out_bufs"]) as out_pool,
            tc.tile_pool(name="yst", bufs=cfg["y_bufs"]) as y_pool,
            tc.tile_pool(name="psum", bufs=cfg["psum_bufs"], space="PSUM") as psum_pool,
        ):
            w_dr_sb = wpool.tile([CI, NG, 2, CO], FP8)
            w_bf_sb = wpool.tile([CI, len(BF_TAPS), CO], BF16)
            nc.sync.dma_start(w_dr_sb[:], w_dr_d[:].rearrange("g p two c -> p g two c"))
            nc.sync.dma_start(w_bf_sb[:], w_bf_d[:].rearrange("k p c -> p k c"))
            pools = (xb_pool, x8_pool, out_pool, psum_pool, y_pool)
            for _ in range(static_reps):
                _emit_body(nc, pools, x8_d, xb_d, y_d, w_dr_sb, w_bf_sb, cfg)

    nc.finalize()
    return nc


def prep_weights(weights: np.ndarray):
    bw = np.sign(np.asarray(weights, dtype=np.float32))  # (co, ci, kh, kw)
    w_dr = np.empty((NG, CI, 2, CO), dtype=ml_dtypes.float8_e4m3)
    for g in range(NG):
        for j in range(2):
            w_dr[g, :, j, :] = bw[:, :, g, j].T.astype(ml_dtypes.float8_e4m3)
    w_bf = np.empty((len(BF_TAPS), CI, CO), dtype=ml_dtypes.bfloat16)
    for j, (kh, kw) in enumerate(BF_TAPS):
        w_bf[j] = bw[:, :, kh, kw].T.astype(ml_dtypes.bfloat16)
    return w_dr, w_bf


def prep_x(x: np.ndarray):
    x = np.asarray(x, dtype=np.float32)
    xpad = np.zeros((B, CI, HP, WP), dtype=np.float32)
    xpad[:, :, 1:H + 1, 1:W + 1] = x
    xb = xpad.astype(ml_dtypes.bfloat16)
    # fp8 shifted planes: plane j at (row, c) = xpad[row, c+j]; cols >= WP-j zero
    x8 = np.zeros((B, CI, NG, HP, W8), dtype=ml_dtypes.float8_e4m3)
    xpad8 = xpad.astype(ml_dtypes.float8_e4m3)
    for j in range(2):
        x8[:, :, j, :, 0:WP - j] = xpad8[:, :, :, j:WP]
    return xb, x8


def make_in_maps(x: np.ndarray, weights: np.ndarray) -> list[dict]:
    xb, x8 = prep_x(x)
    w_dr, w_bf = prep_weights(weights)
    return [
        {"xb": xb[i * BPC:(i + 1) * BPC], "x8": x8[i * BPC:(i + 1) * BPC],
         "w_dr": w_dr, "w_bf": w_bf}
        for i in range(N_CORES)
    ]


def _run_once(x, weights) -> np.ndarray:
    nc = build_program()
    in_maps = make_in_maps(x, weights)
    res = run_bass_kernel_spmd(nc, in_maps, core_ids=list(range(N_CORES)))
    return np.concatenate([r["y"] for r in res.results], axis=0)


_SUBPROC_SRC = """
import sys, numpy as np
sys.path.insert(0, sys.argv[1])
import kernel as K
x = np.load(sys.argv[2]); w = np.load(sys.argv[3])
np.save(sys.argv[4], K._run_once(x, w))
"""


def kernel(x, weights) -> np.ndarray:
    x = np.ascontiguousarray(np.asarray(x, dtype=np.float32))
    weights = np.ascontiguousarray(np.asarray(weights, dtype=np.float32))
    try:
        return _run_once(x, weights)
    except Exception as first_exc:
        # Transient device wedges (NRT_EXEC_UNIT_UNRECOVERABLE, mesh desync)
        # poison the in-process PJRT client; only a fresh process recovers.
        import os
        import subprocess
        import sys
        import tempfile
        import time

        last_exc = first_exc
        moddir = os.path.dirname(os.path.abspath(__file__))
        for attempt in range(2):
            time.sleep(10 * (attempt + 1))
            try:
                with tempfile.TemporaryDirectory() as td:
                    xp, wp, yp = (os.path.join(td, f) for f in
                                  ("x.npy", "w.npy", "y.npy"))
                    np.save(xp, x)
                    np.save(wp, weights)
                    subprocess.run(
                        [sys.executable, "-c", _SUBPROC_SRC, moddir, xp, wp, yp],
                        check=True, timeout=900,
                    )
                    return np.load(yp)
            except Exception as e:
                last_exc = e
        raise last_exc
